# revision 9
# baseline (speedup 1.0000x reference)
"""GAT layer (nn_GATLayer) Trainium2 Bass kernel.

Math (reference):
    h  = X @ W                                     # [N, D]
    s1 = h @ a[:D, 0] ; s2 = h @ a[D:, 0]          # [N]
    e  = exp(leaky_relu(s1[i] + s2[j], 0.2)) * (Adj != 0)
    out = (e / e.sum(axis=1, keepdims=True)) @ h

Kernel decomposition (per core, rows i in its block of 1024):
    z = s1[i] + s2[j]
    exp(lrelu(z)) = exp(0.2 z) * max(exp(0.8 z), 1)
    exp(a z) = exp(a s1[i]) * exp(a s2[j])   (rank-1)
  so with u2=exp(0.2 s1), v2=exp(0.2 s2), u3=exp(0.8 s1), v3=exp(0.8 s2):
    e_T[j, i] = (Adj_T[j, i] * u2[i] * v2[j]) * max(u3[i] * v3[j], 1)
  The (Adj_T * u2[i]) factor comes out of the PE by multiplying each
  128x128 Adj block with diag(u2) (a regular matmul that also transposes),
  v2[j] is a per-partition scalar fused into a DVE scalar_tensor_tensor,
  and the max() factor is one GpSimd tensor_scalar on a broadcast tile.
  Row sums ride along the main matmul as a ones-column appended to h.

Sharding: rows of X/Adj across 8 cores; W/a replicated; h block computed
locally then AllGathered through DRAM so every core has the full h.
"""

import sys
from contextlib import ExitStack

for _p in ("/opt/trn_rl_repo", "/root/.axon_site/_ro/trn_rl_repo"):
    if _p not in sys.path:
        sys.path.insert(0, _p)

import numpy as np

import concourse.bacc as bacc
import concourse.bass as bass
import concourse.mybir as mybir
from concourse import tile
from concourse.bass import ts
from concourse.bass_utils import run_bass_kernel_spmd
from concourse.masks import make_identity

F32 = mybir.dt.float32
I32 = mybir.dt.int32
AF = mybir.ActivationFunctionType
OP = mybir.AluOpType

N = 8192          # nodes
K = 512           # in dim
D = 64            # out dim
NCORES = 8
NB = N // NCORES  # 1024 rows per core
JC = N // 128     # 64 j-chunks
IC = NB // 128    # 8 i-chunks per core
ALPHA = 0.2


def gat_kernel(tc, out_ap, x_ap, adj_ap, w_ap, a_ap):
    """Build the per-core program. All APs are DRAM access patterns:
    out [NB, D] f32, x [NB, K] f32, adj [NB, N] i32, w [K, D] f32, a [2D, 1] f32.
    """
    nc = tc.nc
    octx = ExitStack()

    constp = octx.enter_context(tc.tile_pool(name="const", bufs=1))
    dramp = octx.enter_context(tc.tile_pool(name="dram", bufs=1, space="DRAM"))

    # ---------------- constants ----------------
    eye = constp.tile([128, 128], F32)
    make_identity(nc, eye[:])
    ones_row = constp.tile([1, 128], F32)
    nc.vector.memset(ones_row[:], 1.0)

    # a broadcast across partitions: ab[:, 0:64] = a1, ab[:, 64:128] = a2
    a_row = constp.tile([1, 2 * D], F32)
    nc.sync.dma_start(a_row[:], a_ap.rearrange("d one -> one d"))

    with tc.tile_pool(name="pre_ps", bufs=1, space="PSUM") as pre_ps, \
         tc.tile_pool(name="pre_sb", bufs=3) as pre_sb:
        ab_ps = pre_ps.tile([128, 2 * D], F32)
        nc.tensor.matmul(ab_ps[:], lhsT=ones_row[:], rhs=a_row[:], start=True, stop=True)
        ab = constp.tile([128, 2 * D], F32)
        nc.vector.tensor_copy(ab[:], ab_ps[:])

        # ---------------- h = X @ W for own block ----------------
        wr = constp.tile([128, 4, D], F32)
        nc.sync.dma_start(wr[:], w_ap.rearrange("(kc p) d -> p kc d", p=128))

        h_shard = dramp.tile([NB, D], F32)
        s1_all = constp.tile([128, IC], F32)
        x3 = x_ap.rearrange("(t p) k -> p t k", p=128)
        junk1 = constp.tile([128, D], F32)
        for t in range(IC):
            xs = pre_sb.tile([128, K], F32, tag="xs")
            nc.sync.dma_start(xs[:], x3[:, t, :])
            xt_ps = pre_ps.tile([128, 4, 128], F32, tag="xt_ps")
            for kc in range(4):
                nc.tensor.transpose(xt_ps[:, kc, :], xs[:, ts(kc, 128)], eye[:])
            xt = pre_sb.tile([128, 4, 128], F32, tag="xt")
            nc.vector.tensor_copy(xt[:], xt_ps[:])
            h_ps = pre_ps.tile([128, D], F32, tag="h_ps")
            for kc in range(4):
                nc.tensor.matmul(h_ps[:], lhsT=xt[:, kc, :], rhs=wr[:, kc, :],
                                 start=(kc == 0), stop=(kc == 3))
            h_sb = pre_sb.tile([128, D], F32, tag="h_sb")
            nc.scalar.copy(h_sb[:], h_ps[:])
            nc.sync.dma_start(h_shard[ts(t, 128), :], h_sb[:])
            # s1 for own rows (pre-allgather, so indices stay core-local)
            nc.vector.scalar_tensor_tensor(junk1[:], h_sb[:], 1.0, ab[:, 0:D],
                                           OP.bypass, OP.mult,
                                           accum_out=s1_all[:, t:t + 1])

        # ---------------- AllGather h ----------------
        h_full = dramp.tile([N, D], F32)
        nc.gpsimd.collective_compute(
            "AllGather", OP.bypass,
            replica_groups=[list(range(NCORES))],
            ins=[h_shard.opt()],
            outs=[h_full.opt()],
        )

        # h_aug[p, c, 0:64] = h[c*128+p, :], h_aug[p, c, 64] = 1.0
        h_aug = constp.tile([128, JC, D + 1], F32)
        nc.vector.memset(h_aug[:], 1.0)
        nc.sync.dma_start(h_aug[:, :, 0:D], h_full[:].rearrange("(c p) d -> p c d", p=128))

        # s2 over all nodes
        s2_all = constp.tile([128, JC], F32)
        junk2 = constp.tile([128, D], F32)
        for c in range(JC):
            nc.vector.scalar_tensor_tensor(junk2[:], h_aug[:, c, 0:D], 1.0,
                                           ab[:, D:2 * D], OP.bypass, OP.mult,
                                           accum_out=s2_all[:, c:c + 1])

        # exp factors
        v2 = constp.tile([128, JC], F32)
        nc.scalar.activation(v2[:], s2_all[:], AF.Exp, scale=ALPHA)
        v3 = constp.tile([128, JC], F32)
        nc.scalar.activation(v3[:], s2_all[:], AF.Exp, scale=1.0 - ALPHA)
        u2 = constp.tile([128, IC], F32)
        nc.scalar.activation(u2[:], s1_all[:], AF.Exp, scale=ALPHA)
        u3 = constp.tile([128, IC], F32)
        nc.scalar.activation(u3[:], s1_all[:], AF.Exp, scale=1.0 - ALPHA)

        # diag(u2[ic]) tiles for the transpose-and-scale matmuls
        dg = constp.tile([128, IC, 128], F32)
        for ic in range(IC):
            nc.vector.tensor_scalar(dg[:, ic, :], eye[:], u2[:, ic:ic + 1], None, OP.mult)

        # u3 broadcast along free dim: u3_b[p, ic, q] = u3[q, ic] (= u3 of node ic*128+q)
        u3t_ps = pre_ps.tile([IC, 128], F32, tag="u3t_ps")
        nc.tensor.transpose(u3t_ps[:], u3[:], eye[:])
        u3t = pre_sb.tile([IC, 128], F32, tag="u3t")
        nc.vector.tensor_copy(u3t[:], u3t_ps[:])
        u3row = pre_sb.tile([1, NB], F32, tag="u3row")
        nc.sync.dma_start(u3row[:], u3t[:])  # flatten partitions into one row
        u3b_ps = pre_ps.tile([128, IC, 128], F32, tag="u3b_ps")
        for hh in range(2):
            nc.tensor.matmul(u3b_ps[:, 4 * hh:4 * hh + 4, :], lhsT=ones_row[:],
                             rhs=u3row[:, ts(hh, 512)], start=True, stop=True)
        u3_b = constp.tile([128, IC, 128], F32)
        nc.vector.tensor_copy(u3_b[:], u3b_ps[:])

    # ---------------- main loop over j-chunks ----------------
    adj3 = adj_ap.rearrange("(ic p) (jc j) -> p ic jc j", p=128, j=128)

    out_ps_pool = octx.enter_context(tc.tile_pool(name="out_ps", bufs=1, space="PSUM"))
    outa_ps = out_ps_pool.tile([D + 1, 512], F32)
    outb_ps = out_ps_pool.tile([D + 1, 512], F32)

    with tc.tile_pool(name="adj_i", bufs=3) as adj_i_pool, \
         tc.tile_pool(name="adj_f", bufs=2) as adj_f_pool, \
         tc.tile_pool(name="p2", bufs=2, space="PSUM") as p2_pool, \
         tc.tile_pool(name="qe", bufs=3) as qe_pool:
        for jc in range(JC):
            adji = adj_i_pool.tile([128, IC, 128], I32, tag="adji")
            nc.sync.dma_start(adji[:], adj3[:, :, jc, :])
            adjf = adj_f_pool.tile([128, IC, 128], F32, tag="adjf")
            nc.scalar.copy(adjf[:], adji[:])

            p2_ps = p2_pool.tile([128, IC, 128], F32, tag="p2")
            for ic in range(IC):
                # p2[:, ic, q] over partitions j: sum_i adj[i, j] * diag_u2[i, q]
                nc.tensor.matmul(p2_ps[:, ic, :], lhsT=adjf[:, ic, :], rhs=dg[:, ic, :],
                                 start=True, stop=True)

            q = qe_pool.tile([128, IC, 128], F32, tag="q")
            nc.gpsimd.tensor_scalar(q[:], u3_b[:], v3[:, jc:jc + 1], 1.0, OP.mult, OP.max)

            e_t = qe_pool.tile([128, IC, 128], F32, tag="e_t")
            nc.vector.scalar_tensor_tensor(e_t[:], p2_ps[:], v2[:, jc:jc + 1], q[:],
                                           OP.mult, OP.mult)

            first, last = (jc == 0), (jc == JC - 1)
            nc.tensor.matmul(outa_ps[:], lhsT=h_aug[:, jc, :], rhs=e_t[:, 0:4, :],
                             start=first, stop=last)
            nc.tensor.matmul(outb_ps[:], lhsT=h_aug[:, jc, :], rhs=e_t[:, 4:8, :],
                             start=first, stop=last)

    # ---------------- normalize + transpose back + store ----------------
    with tc.tile_pool(name="post_sb", bufs=2) as post_sb, \
         tc.tile_pool(name="post_ps", bufs=2, space="PSUM") as post_ps:
        for half, o_ps in enumerate((outa_ps, outb_ps)):
            osb = post_sb.tile([D + 1, 512], F32, tag="osb")
            nc.scalar.copy(osb[:], o_ps[:])
            for b in range(4):
                o2_ps = post_ps.tile([128, D + 1], F32, tag="o2")
                nc.tensor.transpose(o2_ps[:], osb[:, ts(b, 128)], eye[0:D + 1, 0:D + 1])
                rcp = post_sb.tile([128, 1], F32, tag="rcp")
                nc.vector.reciprocal(rcp[:], o2_ps[:, D:D + 1])
                fin = post_sb.tile([128, D], F32, tag="fin")
                nc.vector.tensor_scalar(fin[:], o2_ps[:, 0:D], rcp[:], None, OP.mult)
                nc.sync.dma_start(out_ap[bass.ds(half * 512 + b * 128, 128), :], fin[:])

    octx.close()


_BUILT = None


def _build():
    global _BUILT
    if _BUILT is not None:
        return _BUILT
    nc = bacc.Bacc("TRN2", target_bir_lowering=False, debug=False,
                   enable_asserts=False, num_devices=NCORES)
    x = nc.dram_tensor("X_blk", [NB, K], F32, kind="ExternalInput")
    adj = nc.dram_tensor("Adj_blk", [NB, N], I32, kind="ExternalInput")
    w = nc.dram_tensor("W", [K, D], F32, kind="ExternalInput")
    a = nc.dram_tensor("a", [2 * D, 1], F32, kind="ExternalInput")
    out = nc.dram_tensor("out", [NB, D], F32, kind="ExternalOutput")
    with tile.TileContext(nc) as tc:
        gat_kernel(tc, out.ap(), x.ap(), adj.ap(), w.ap(), a.ap())
    nc.compile()
    _BUILT = nc
    return nc


def kernel(X, Adj, W, a, _trace=False):
    X = np.ascontiguousarray(np.asarray(X, dtype=np.float32))
    Adj = np.ascontiguousarray(np.asarray(Adj, dtype=np.int32))
    W = np.ascontiguousarray(np.asarray(W, dtype=np.float32))
    a = np.ascontiguousarray(np.asarray(a, dtype=np.float32))

    nc = _build()
    in_maps = [
        {
            "X_blk": X[c * NB:(c + 1) * NB],
            "Adj_blk": Adj[c * NB:(c + 1) * NB],
            "W": W,
            "a": a,
        }
        for c in range(NCORES)
    ]
    res = run_bass_kernel_spmd(nc, in_maps, core_ids=list(range(NCORES)),
                               trace=_trace)
    out = np.concatenate([res.results[c]["out"] for c in range(NCORES)], axis=0)
    if _trace:
        kernel.last_results = res
    return out


# revision 12
# speedup vs baseline: 6.6180x; 6.6180x over previous
"""GAT layer (nn_GATLayer) Trainium2 Bass kernel.

Math (reference):
    h  = X @ W                                     # [N, D]
    s1 = h @ a[:D, 0] ; s2 = h @ a[D:, 0]          # [N]
    e  = exp(leaky_relu(s1[i] + s2[j], 0.2)) * (Adj != 0)
    out = (e / e.sum(axis=1, keepdims=True)) @ h

Kernel decomposition (per core, rows i in its block of 1024):
    z = s1[i] + s2[j]
    exp(lrelu(z)) = exp(0.2 z) * max(exp(0.8 z), 1)
    exp(a z) = exp(a s1[i]) * exp(a s2[j])   (rank-1)
  so with u2=exp(0.2 s1), v2=exp(0.2 s2), u3=exp(0.8 s1), v3=exp(0.8 s2):
    e_T[j, i] = (Adj_T[j, i] * u2[i] * v2[j]) * max(u3[i] * v3[j], 1)
  The (Adj_T * u2[i]) factor comes out of the PE by multiplying each
  128x128 Adj block with diag(u2) (a regular matmul that also transposes),
  v2[j] is a per-partition scalar fused into a DVE scalar_tensor_tensor,
  and the max() factor is one GpSimd tensor_scalar on a broadcast tile.
  Row sums ride along the main matmul as a ones-column appended to h.

Sharding: rows of X/Adj across 8 cores; W/a replicated; h block computed
locally then AllGathered through DRAM so every core has the full h.
"""

import sys
from contextlib import ExitStack

for _p in ("/opt/trn_rl_repo", "/root/.axon_site/_ro/trn_rl_repo"):
    if _p not in sys.path:
        sys.path.insert(0, _p)

import numpy as np

import concourse.bacc as bacc
import concourse.bass as bass
import concourse.mybir as mybir
from concourse import tile
from concourse.bass import ts
from concourse.bass_utils import run_bass_kernel_spmd
from concourse.masks import make_identity

F32 = mybir.dt.float32
I32 = mybir.dt.int32
AF = mybir.ActivationFunctionType
OP = mybir.AluOpType

N = 8192          # nodes
K = 512           # in dim
D = 64            # out dim
NCORES = 8
NB = N // NCORES  # 1024 rows per core
JC = N // 128     # 64 j-chunks
IC = NB // 128    # 8 i-chunks per core
ALPHA = 0.2


def gat_kernel(tc, out_ap, x_ap, adj_ap, w_ap, a_ap, repeat=1):
    """Build the per-core program. All APs are DRAM access patterns:
    out [NB, D] f32, x [NB, K] f32, adj [NB, N] i32, w [K, D] f32, a [2D, 1] f32.
    """
    nc = tc.nc
    octx = ExitStack()

    constp = octx.enter_context(tc.tile_pool(name="const", bufs=1))
    dramp = octx.enter_context(tc.tile_pool(name="dram", bufs=1, space="DRAM"))

    # ---------------- constants ----------------
    eye = constp.tile([128, 128], F32)
    make_identity(nc, eye[:])
    ones_row = constp.tile([1, 128], F32)
    nc.vector.memset(ones_row[:], 1.0)

    # a broadcast across partitions: ab[:, 0:64] = a1, ab[:, 64:128] = a2
    a_row = constp.tile([1, 2 * D], F32)
    nc.sync.dma_start(a_row[:], a_ap.rearrange("d one -> one d"))

    with tc.tile_pool(name="pre_ps", bufs=1, space="PSUM") as pre_ps, \
         tc.tile_pool(name="pre_sb", bufs=3) as pre_sb:
        ab_ps = pre_ps.tile([128, 2 * D], F32)
        nc.tensor.matmul(ab_ps[:], lhsT=ones_row[:], rhs=a_row[:], start=True, stop=True)
        ab = constp.tile([128, 2 * D], F32)
        nc.vector.tensor_copy(ab[:], ab_ps[:])

        # ---------------- h = X @ W for own block ----------------
        wr = constp.tile([128, 4, D], F32)
        nc.sync.dma_start(wr[:], w_ap.rearrange("(kc p) d -> p kc d", p=128))

        h_shard = dramp.tile([NB, D], F32)
        s1_all = constp.tile([128, IC], F32)
        x3 = x_ap.rearrange("(t p) k -> p t k", p=128)
        junk1 = constp.tile([128, D], F32)
        for t in range(IC):
            xs = pre_sb.tile([128, K], F32, tag="xs")
            nc.sync.dma_start(xs[:], x3[:, t, :])
            xt_ps = pre_ps.tile([128, 4, 128], F32, tag="xt_ps")
            for kc in range(4):
                nc.tensor.transpose(xt_ps[:, kc, :], xs[:, ts(kc, 128)], eye[:])
            xt = pre_sb.tile([128, 4, 128], F32, tag="xt")
            nc.vector.tensor_copy(xt[:], xt_ps[:])
            h_ps = pre_ps.tile([128, D], F32, tag="h_ps")
            for kc in range(4):
                nc.tensor.matmul(h_ps[:], lhsT=xt[:, kc, :], rhs=wr[:, kc, :],
                                 start=(kc == 0), stop=(kc == 3))
            h_sb = pre_sb.tile([128, D], F32, tag="h_sb")
            nc.scalar.copy(h_sb[:], h_ps[:])
            nc.sync.dma_start(h_shard[ts(t, 128), :], h_sb[:])
            # s1 for own rows (pre-allgather, so indices stay core-local)
            nc.vector.scalar_tensor_tensor(junk1[:], h_sb[:], 1.0, ab[:, 0:D],
                                           OP.bypass, OP.mult,
                                           accum_out=s1_all[:, t:t + 1])

        # ---------------- AllGather h ----------------
        h_full = dramp.tile([N, D], F32)
        nc.gpsimd.collective_compute(
            "AllGather", OP.bypass,
            replica_groups=[list(range(NCORES))],
            ins=[h_shard.opt()],
            outs=[h_full.opt()],
        )

        # h_aug[p, c, 0:64] = h[c*128+p, :], h_aug[p, c, 64] = 1.0
        h_aug = constp.tile([128, JC, D + 1], F32)
        nc.vector.memset(h_aug[:], 1.0)
        nc.sync.dma_start(h_aug[:, :, 0:D], h_full[:].rearrange("(c p) d -> p c d", p=128))

        # s2 over all nodes
        s2_all = constp.tile([128, JC], F32)
        junk2 = constp.tile([128, D], F32)
        for c in range(JC):
            nc.vector.scalar_tensor_tensor(junk2[:], h_aug[:, c, 0:D], 1.0,
                                           ab[:, D:2 * D], OP.bypass, OP.mult,
                                           accum_out=s2_all[:, c:c + 1])

        # exp factors
        v2 = constp.tile([128, JC], F32)
        nc.scalar.activation(v2[:], s2_all[:], AF.Exp, scale=ALPHA)
        v3 = constp.tile([128, JC], F32)
        nc.scalar.activation(v3[:], s2_all[:], AF.Exp, scale=1.0 - ALPHA)
        u2 = constp.tile([128, IC], F32)
        nc.scalar.activation(u2[:], s1_all[:], AF.Exp, scale=ALPHA)
        u3 = constp.tile([128, IC], F32)
        nc.scalar.activation(u3[:], s1_all[:], AF.Exp, scale=1.0 - ALPHA)

        # diag(u2[ic]) tiles for the transpose-and-scale matmuls
        dg = constp.tile([128, IC, 128], F32)
        for ic in range(IC):
            nc.vector.tensor_scalar(dg[:, ic, :], eye[:], u2[:, ic:ic + 1], None, OP.mult)

        # u3 broadcast along free dim: u3_b[p, ic, q] = u3[q, ic] (= u3 of node ic*128+q)
        u3t_ps = pre_ps.tile([IC, 128], F32, tag="u3t_ps")
        nc.tensor.transpose(u3t_ps[:], u3[:], eye[:])
        u3t = pre_sb.tile([IC, 128], F32, tag="u3t")
        nc.vector.tensor_copy(u3t[:], u3t_ps[:])
        u3row = pre_sb.tile([1, NB], F32, tag="u3row")
        nc.sync.dma_start(u3row[:], u3t[:])  # flatten partitions into one row
        u3b_ps = pre_ps.tile([128, IC, 128], F32, tag="u3b_ps")
        for hh in range(2):
            nc.tensor.matmul(u3b_ps[:, 4 * hh:4 * hh + 4, :], lhsT=ones_row[:],
                             rhs=u3row[:, ts(hh, 512)], start=True, stop=True)
        u3_b = constp.tile([128, IC, 128], F32)
        nc.vector.tensor_copy(u3_b[:], u3b_ps[:])

    # ---------------- main loop over j-chunks ----------------
    adj3 = adj_ap.rearrange("(ic p) (jc j) -> p ic jc j", p=128, j=128)

    out_ps_pool = octx.enter_context(tc.tile_pool(name="out_ps", bufs=1, space="PSUM"))
    outa_ps = out_ps_pool.tile([D + 1, 512], F32)
    outb_ps = out_ps_pool.tile([D + 1, 512], F32)

    with tc.tile_pool(name="adj_i", bufs=3) as adj_i_pool, \
         tc.tile_pool(name="adj_f", bufs=2) as adj_f_pool, \
         tc.tile_pool(name="p2", bufs=2, space="PSUM") as p2_pool, \
         tc.tile_pool(name="qe", bufs=3) as qe_pool:
        for rep in range(repeat):
            for jc in range(JC):
                adji = adj_i_pool.tile([128, IC, 128], I32, tag="adji")
                nc.sync.dma_start(adji[:], adj3[:, :, jc, :])
                adjf = adj_f_pool.tile([128, IC, 128], F32, tag="adjf")
                nc.scalar.copy(adjf[:], adji[:])

                p2_ps = p2_pool.tile([128, IC, 128], F32, tag="p2")
                for ic in range(IC):
                    # p2[:, ic, q] over partitions j: sum_i adj[i, j] * diag_u2[i, q]
                    nc.tensor.matmul(p2_ps[:, ic, :], lhsT=adjf[:, ic, :],
                                     rhs=dg[:, ic, :], start=True, stop=True)

                q = qe_pool.tile([128, IC, 128], F32, tag="q")
                nc.gpsimd.tensor_scalar(q[:], u3_b[:], v3[:, jc:jc + 1], 1.0,
                                        OP.mult, OP.max)

                e_t = qe_pool.tile([128, IC, 128], F32, tag="e_t")
                nc.vector.scalar_tensor_tensor(e_t[:], p2_ps[:], v2[:, jc:jc + 1],
                                               q[:], OP.mult, OP.mult)

                first = (jc == 0) and (rep == 0)
                last = (jc == JC - 1) and (rep == repeat - 1)
                nc.tensor.matmul(outa_ps[:], lhsT=h_aug[:, jc, :], rhs=e_t[:, 0:4, :],
                                 start=first, stop=last)
                nc.tensor.matmul(outb_ps[:], lhsT=h_aug[:, jc, :], rhs=e_t[:, 4:8, :],
                                 start=first, stop=last)

    # ---------------- normalize + transpose back + store ----------------
    with tc.tile_pool(name="post_sb", bufs=2) as post_sb, \
         tc.tile_pool(name="post_ps", bufs=2, space="PSUM") as post_ps:
        for half, o_ps in enumerate((outa_ps, outb_ps)):
            osb = post_sb.tile([D + 1, 512], F32, tag="osb")
            nc.scalar.copy(osb[:], o_ps[:])
            for b in range(4):
                o2_ps = post_ps.tile([128, D + 1], F32, tag="o2")
                nc.tensor.transpose(o2_ps[:], osb[:, ts(b, 128)], eye[0:D + 1, 0:D + 1])
                rcp = post_sb.tile([128, 1], F32, tag="rcp")
                nc.vector.reciprocal(rcp[:], o2_ps[:, D:D + 1])
                fin = post_sb.tile([128, D], F32, tag="fin")
                nc.vector.tensor_scalar(fin[:], o2_ps[:, 0:D], rcp[:], None, OP.mult)
                nc.sync.dma_start(out_ap[bass.ds(half * 512 + b * 128, 128), :], fin[:])

    octx.close()


_BUILT = {}


def _build(repeat=1):
    if repeat in _BUILT:
        return _BUILT[repeat]
    nc = bacc.Bacc("TRN2", target_bir_lowering=False, debug=False,
                   enable_asserts=False, num_devices=NCORES)
    x = nc.dram_tensor("X_blk", [NB, K], F32, kind="ExternalInput")
    adj = nc.dram_tensor("Adj_blk", [NB, N], I32, kind="ExternalInput")
    w = nc.dram_tensor("W", [K, D], F32, kind="ExternalInput")
    a = nc.dram_tensor("a", [2 * D, 1], F32, kind="ExternalInput")
    out = nc.dram_tensor("out", [NB, D], F32, kind="ExternalOutput")
    with tile.TileContext(nc) as tc:
        gat_kernel(tc, out.ap(), x.ap(), adj.ap(), w.ap(), a.ap(), repeat=repeat)
    nc.compile()
    _BUILT[repeat] = nc
    return nc


def kernel(X, Adj, W, a, _trace=False):
    X = np.ascontiguousarray(np.asarray(X, dtype=np.float32))
    Adj = np.ascontiguousarray(np.asarray(Adj, dtype=np.int32))
    W = np.ascontiguousarray(np.asarray(W, dtype=np.float32))
    a = np.ascontiguousarray(np.asarray(a, dtype=np.float32))

    nc = _build()
    in_maps = [
        {
            "X_blk": X[c * NB:(c + 1) * NB],
            "Adj_blk": Adj[c * NB:(c + 1) * NB],
            "W": W,
            "a": a,
        }
        for c in range(NCORES)
    ]
    res = run_bass_kernel_spmd(nc, in_maps, core_ids=list(range(NCORES)),
                               trace=_trace)
    out = np.concatenate([res.results[c]["out"] for c in range(NCORES)], axis=0)
    if _trace:
        kernel.last_results = res
    return out
